# revision 11
# baseline (speedup 1.0000x reference)
"""Trainium2 Bass kernel for nn_AttentionV4 (patch attention, 8 heads on 8 cores).

Pipeline per core (= per head h):
  - The 1x1 qkv conv + depthwise 3x3 conv are fused into one dense 3x3 conv,
    expressed as a single matmul over a 6x6-windowed patch basis:
      Q/K/V[r, n] = sum_kappa W4[kappa, r] * Xp[kappa, n],
    kappa = (ph, pw, c) in [6,6,48] (1728, chunked 18 x 96), n = interior
    patch (64x64 grid = 4096; boundary patches of the stride-4 pad-4 unfold
    are exactly zero and are handled analytically).
  - unfold/fold with kernel=stride=4 are pure reshapes; Xp windows are read
    straight out of a host-prepared block-layout image xb[hm,wm,c,hq,wq].
  - l2-normalize Q (x temperature) and K per column, A = Qn^T Kn in [-1,1],
    so softmax needs no max subtraction: E = exp(A), Z = rowsum(E) + 260
    (260 = number of zero boundary K columns, each contributing exp(0)).
  - out = (V/Z) @ E accumulated over 32 row-tiles of 128.
  - fold + AllToAll routes output stripe s to core s; each core applies the
    final 48x48 projection to its own 32-row stripe.
"""
import sys
import types

sys.path.insert(0, "/opt/trn_rl_repo")

import numpy as np

# ---------------------------------------------------------------- constants
C = 48          # image channels
CH = 6          # channels per head
NH = 8          # heads == cores
GN = 64         # interior patch grid
N = GN * GN     # 4096 interior patches
M96 = 96        # rows of a head matrix (6ch * 4 * 4)
NKAP = 1728     # 36 windows * 48 channels
NCHUNK = 18     # kappa chunks of 96
ZCORR = 260.0   # 4356 - 4096 zero K-columns, exp(0) each
NPIECE = 16     # front-end N pieces (4 patch rows, 256 patches each)
NCORES = 8

# chunk table: pairs of (hm, wm), (hm, wm+1) groups within one (dh, dw) class
def _chunk_table():
    info = []
    for dh, dw in [(0, 0), (0, 1), (1, 0), (1, 1)]:
        groups = [(hm, wm)
                  for hm in range(4 if dh == 0 else 2)
                  for wm in range(4 if dw == 0 else 2)]
        for i in range(0, len(groups), 2):
            (hm0, wm0), (hm1, wm1) = groups[i], groups[i + 1]
            assert hm0 == hm1 and wm1 == wm0 + 1
            info.append((dh, dw, hm0, wm0))
    assert len(info) == NCHUNK
    return info

CHUNKS = _chunk_table()

# kappa order implied by the chunk table (ph, pw, c), c fastest
def _kappa_phpw():
    phs, pws = [], []
    for dh, dw, hm, wm0 in CHUNKS:
        for wm in (wm0, wm0 + 1):
            ph, pw = dh * 4 + hm, dw * 4 + wm
            phs += [ph] * C
            pws += [pw] * C
    return np.array(phs), np.array(pws)

_PHS, _PWS = _kappa_phpw()
_CS = np.tile(np.arange(C), NKAP // C)

# ---------------------------------------------------------------- host prep

def _build_xb(x):
    """Block layout of the (+1,+3)-padded image: xb[hm, wm, c, hq, wq]."""
    xpad = np.zeros((C, 260, 260), np.float32)
    xpad[:, 1:257, 1:257] = x[0]
    return np.ascontiguousarray(
        xpad.reshape(C, 65, 4, 65, 4).transpose(2, 4, 0, 1, 3))


def _build_w4(h, w_qkv, w_dw):
    """Fused (1x1 conv + dw3x3) weights in the kappa basis: [1728, 288]."""
    kh = np.arange(4)
    dy = _PHS[:, None] - kh[None, :]            # [1728, 4]
    dx = _PWS[:, None] - kh[None, :]
    my = (dy >= 0) & (dy < 3)
    mx = (dx >= 0) & (dx < 3)
    dyc = np.clip(dy, 0, 2)
    dxc = np.clip(dx, 0, 2)
    w4 = np.zeros((NKAP, 3, CH, 4, 4), np.float32)
    for sel in range(3):
        for cl in range(CH):
            o = sel * C + CH * h + cl
            wd = w_dw[o, 0]
            taps = (wd[dyc[:, :, None], dxc[:, None, :]]
                    * my[:, :, None] * mx[:, None, :])
            w4[:, sel, cl] = w_qkv[o, _CS][:, None, None] * taps
    return w4.reshape(NKAP, 288)


# ---------------------------------------------------------------- program

_PROG = None

def _build_program():
    import antenv  # noqa: F401
    if "antenv.axon_hooks" not in sys.modules:
        holder = {}
        m = types.ModuleType("antenv.axon_hooks")
        m.set_axon_ntff_profile_hook = lambda hk: holder.__setitem__("h", hk)
        m.get_axon_ntff_profile_hook = lambda: holder.get("h")
        sys.modules["antenv.axon_hooks"] = m
        antenv.axon_hooks = m
        try:
            from trn_agent_boot.trn_boot import _ntff_profile_via_ctypes
            m.set_axon_ntff_profile_hook(
                _ntff_profile_via_ctypes("/opt/axon/libaxon_pjrt.so"))
        except Exception:
            pass

    import concourse.bass as bass
    import concourse.tile as tile
    import concourse.mybir as mybir
    from contextlib import ExitStack

    F32 = mybir.dt.float32
    AF = mybir.ActivationFunctionType

    nc = bass.Bass("TRN2", num_devices=NCORES)

    xb_h = nc.dram_tensor("xb", [4, 4, C, 65, 65], F32, kind="ExternalInput")
    w4_h = nc.dram_tensor("w4", [NKAP, 288], F32, kind="ExternalInput")
    vcol_h = nc.dram_tensor("vcol", [M96, 2], F32, kind="ExternalInput")
    wpt_h = nc.dram_tensor("wpt", [C, C], F32, kind="ExternalInput")
    id96_h = nc.dram_tensor("id96", [M96, M96], F32, kind="ExternalInput")
    ones_h = nc.dram_tensor("onesrow", [1, M96], F32, kind="ExternalInput")
    y_h = nc.dram_tensor("y", [C, 8192], F32, kind="ExternalOutput")
    cc_in = nc.dram_tensor("cc_in", [C, 8192], F32)
    cc_out = nc.dram_tensor("cc_out", [C, 8192], F32)

    with tile.TileContext(nc) as tc, ExitStack() as ctx:
        const = ctx.enter_context(tc.tile_pool(name="const", bufs=1))
        w4_sb = const.tile([M96, NCHUNK * 288], F32)
        nc.sync.dma_start(
            w4_sb[:].rearrange("p (k o) -> p k o", k=NCHUNK),
            w4_h[:].rearrange("(k p) o -> p k o", p=M96))
        vcol_sb = const.tile([M96, 2], F32)
        nc.sync.dma_start(vcol_sb[:], vcol_h[:])
        wpt_sb = const.tile([C, C], F32)
        nc.sync.dma_start(wpt_sb[:], wpt_h[:])
        id96_sb = const.tile([M96, M96], F32)
        nc.sync.dma_start(id96_sb[:], id96_h[:])
        ones_sb = const.tile([1, M96], F32)
        nc.sync.dma_start(ones_sb[:], ones_h[:])

        persist = ctx.enter_context(tc.tile_pool(name="persist", bufs=1))
        qn = persist.tile([M96, N], F32)
        kn = persist.tile([M96, N], F32)
        vt = persist.tile([128, 32 * M96], F32)
        out_acc = persist.tile([M96, N], F32)
        zacc = persist.tile([128, 128], F32)

        # ---------------- front end: Q/K/V + column sumsq ----------------
        ctx2 = tc.tile_pool(name="fe_persist", bufs=1)
        fep = ctx2.__enter__()
        vn = fep.tile([M96, N], F32)
        rq_row = fep.tile([1, N], F32)
        rk_row = fep.tile([1, N], F32)
        with tc.tile_pool(name="fe_xp", bufs=2) as xp_pool, \
             tc.tile_pool(name="fe_ps", bufs=3, space="PSUM") as fps, \
             tc.tile_pool(name="fe_ssps", bufs=2, space="PSUM") as ssps, \
             tc.tile_pool(name="fe_tmp", bufs=2) as fe_tmp:
            for p in range(NPIECE):
                r0 = 4 * p
                xp_t = xp_pool.tile([M96, NCHUNK, 5, 65], F32)
                for k, (dh, dw, hm, wm0) in enumerate(CHUNKS):
                    nc.sync.dma_start(
                        xp_t[:, k, :, :],
                        xb_h[hm, wm0:wm0 + 2, :, r0:r0 + 5, :]
                        .rearrange("a c r w -> (a c) r w"))
                cols = slice(256 * p, 256 * (p + 1))
                for sel, dst in ((0, qn), (1, kn), (2, vn)):
                    ps = fps.tile([M96, 256], F32)
                    for k, (dh, dw, hm, wm0) in enumerate(CHUNKS):
                        nc.tensor.matmul(
                            ps[:],
                            lhsT=w4_sb[:, 288 * k + M96 * sel:
                                       288 * k + M96 * (sel + 1)],
                            rhs=xp_t[:, k, dh:dh + 4, dw:dw + 64],
                            start=(k == 0), stop=(k == NCHUNK - 1))
                    nc.vector.tensor_copy(dst[:, cols], ps[:])
                    if sel < 2:
                        sq = fe_tmp.tile([M96, 256], F32)
                        nc.scalar.activation(sq[:], ps[:], AF.Square)
                        ssp = ssps.tile([1, 256], F32)
                        nc.tensor.matmul(
                            ssp[:], lhsT=vcol_sb[:, sel:sel + 1], rhs=sq[:],
                            start=True, stop=True)
                        row = rq_row if sel == 0 else rk_row
                        nc.vector.tensor_copy(row[0:1, cols], ssp[:])

        # ---------------- normalize Q (x temp) and K ----------------
        with tc.tile_pool(name="nrm", bufs=2) as npool, \
             tc.tile_pool(name="nrm_ps", bufs=2, space="PSUM") as npsum:
            for mt in range(8):
                cols = slice(512 * mt, 512 * (mt + 1))
                for row, dst in ((rq_row, qn), (rk_row, kn)):
                    bp = npsum.tile([M96, 512], F32)
                    nc.tensor.matmul(bp[:], lhsT=ones_sb[:], rhs=row[0:1, cols],
                                     start=True, stop=True)
                    b = npool.tile([M96, 512], F32)
                    nc.vector.reciprocal(b[:], bp[:])
                    nc.scalar.activation(b[:], b[:], AF.Sqrt)
                    nc.vector.tensor_mul(dst[:, cols], dst[:, cols], b[:])

        # ---------------- V^T via PE transpose ----------------
        with tc.tile_pool(name="tp", bufs=2, space="PSUM") as tpool:
            for t in range(32):
                tp = tpool.tile([128, M96], F32)
                nc.tensor.transpose(
                    tp[:], vn[:, 128 * t:128 * (t + 1)], id96_sb[:])
                nc.vector.tensor_copy(vt[:, M96 * t:M96 * (t + 1)], tp[:])
        ctx2.__exit__(None, None, None)

        # ---------------- attention ----------------
        with tc.tile_pool(name="a_ps", bufs=2, space="PSUM") as apsum, \
             tc.tile_pool(name="o_ps", bufs=4, space="PSUM") as opsum, \
             tc.tile_pool(name="e_sb", bufs=6) as epool, \
             tc.tile_pool(name="z_sb", bufs=2) as zpool, \
             tc.tile_pool(name="vts", bufs=8) as vtspool:
            for g in range(8):
                estrips = []
                for tl in range(4):
                    t = 4 * g + tl
                    es = epool.tile([128, N], F32)
                    estrips.append(es)
                    for mp in range(4):
                        pa = apsum.tile([128, 1024], F32)
                        for half in range(2):
                            nc.tensor.matmul(
                                pa[:, 512 * half:512 * (half + 1)],
                                lhsT=qn[:, 128 * t:128 * (t + 1)],
                                rhs=kn[:, 1024 * mp + 512 * half:
                                       1024 * mp + 512 * (half + 1)],
                                start=True, stop=True)
                        col = 4 * t + mp
                        nc.scalar.activation(
                            es[:, 1024 * mp:1024 * (mp + 1)], pa[:], AF.Exp,
                            accum_out=zacc[:, col:col + 1])
                # Z for the group's 4 row-tiles: sum 4 accum cols, +260, 1/x
                zinv = zpool.tile([128, 4], F32)
                nc.vector.tensor_reduce(
                    zinv[:],
                    zacc[:, 16 * g:16 * (g + 1)].rearrange(
                        "p (t m) -> p t m", t=4),
                    axis=mybir.AxisListType.X, op=mybir.AluOpType.add)
                nc.vector.tensor_scalar_add(zinv[:], zinv[:], ZCORR)
                nc.vector.reciprocal(zinv[:], zinv[:])
                vts_tiles = []
                for tl in range(4):
                    t = 4 * g + tl
                    vts = vtspool.tile([128, M96], F32)
                    nc.vector.tensor_scalar_mul(
                        vts[:], vt[:, M96 * t:M96 * (t + 1)],
                        zinv[:, tl:tl + 1])
                    vts_tiles.append(vts)
                for half in range(2):
                    pos = [opsum.tile([M96, 512], F32, tag="pos",
                                      name=f"pos{jj}")
                           for jj in range(4)]
                    for tl in range(4):
                        for jj in range(4):
                            j = 4 * half + jj
                            nc.tensor.matmul(
                                pos[jj][:], lhsT=vts_tiles[tl],
                                rhs=estrips[tl][:, 512 * j:512 * (j + 1)],
                                start=(tl == 0), stop=(tl == 3))
                    for jj in range(4):
                        j = 4 * half + jj
                        cols = slice(512 * j, 512 * (j + 1))
                        if g == 0:
                            nc.vector.tensor_copy(out_acc[:, cols], pos[jj][:])
                        else:
                            nc.vector.tensor_add(
                                out_acc[:, cols], out_acc[:, cols], pos[jj][:])

        # ---------------- fold staging + AllToAll + projection ----------------
        nc.sync.dma_start(
            cc_in[:].rearrange("(s cl) (khw i j) -> (cl khw) s i j",
                               s=8, cl=CH, khw=16, i=8),
            out_acc[:].rearrange("p (s i j) -> p s i j", s=8, i=8))
        nc.gpsimd.collective_compute(
            "AllToAll", mybir.AluOpType.bypass,
            replica_groups=[list(range(NCORES))],
            ins=[cc_in[:]], outs=[cc_out[:]])
        with tc.tile_pool(name="prj", bufs=2) as prj, \
             tc.tile_pool(name="prj_ps", bufs=2, space="PSUM") as prjps, \
             tc.tile_pool(name="yt", bufs=2) as ypool:
            for q in range(16):
                cols = slice(512 * q, 512 * (q + 1))
                fold_t = prj.tile([C, 512], F32)
                nc.sync.dma_start(fold_t[:], cc_out[:, cols])
                pp = prjps.tile([C, 512], F32)
                nc.tensor.matmul(pp[:], lhsT=wpt_sb[:], rhs=fold_t[:],
                                 start=True, stop=True)
                yt = ypool.tile([C, 512], F32)
                nc.vector.tensor_copy(yt[:], pp[:])
                nc.sync.dma_start(y_h[:, cols], yt[:])

    _split_excess_waits(nc)
    return nc


_wsplit_ctr = [0]

def _split_excess_waits(nc, max_waits=1):
    """This walrus build encodes only one sync-wait per instruction; hoist
    extras onto same-engine nops inserted directly before the instruction."""
    import bass_rust
    import concourse.mybir as mybir
    for fn in nc.m.functions:
        for bb in fn.blocks:
            insts = bb.instructions
            out = []
            changed = False
            for inst in insts:
                si = inst.sync_info
                if si is not None and len(si.on_wait) > max_waits:
                    waits = list(si.on_wait)
                    for w in waits[:-max_waits]:
                        _wsplit_ctr[0] += 1
                        nop = bass_rust.InstNoOp(
                            name=f"I-wsplit-{_wsplit_ctr[0]}", ins=[], outs=[])
                        nop.engine = inst.engine
                        nop.sync_info = mybir.SyncInfo(
                            on_wait=[w], on_update=[])
                        out.append(nop)
                    inst.sync_info = mybir.SyncInfo(
                        on_wait=waits[-max_waits:],
                        on_update=list(si.on_update))
                    changed = True
                out.append(inst)
            if changed:
                bb.instructions = out


def _get_program():
    global _PROG
    if _PROG is None:
        _PROG = _build_program()
    return _PROG


# ---------------------------------------------------------------- entry

def kernel(x, w_qkv, w_dw, temperature, w_proj, _trace=False):
    x = np.asarray(x, np.float32)
    w_qkv = np.asarray(w_qkv, np.float32)
    w_dw = np.asarray(w_dw, np.float32)
    temperature = np.asarray(temperature, np.float32)
    w_proj = np.asarray(w_proj, np.float32)

    nc = _get_program()
    from concourse.bass_utils import run_bass_kernel_spmd

    xb = _build_xb(x)
    id96 = np.eye(M96, dtype=np.float32)
    wpt = np.ascontiguousarray(w_proj.T)
    in_maps = []
    for h in range(NH):
        t_h = float(temperature[h, 0, 0])
        vcol = np.empty((M96, 2), np.float32)
        vcol[:, 0] = 1.0 / (t_h * t_h)
        vcol[:, 1] = 1.0
        in_maps.append({
            "xb": xb,
            "w4": _build_w4(h, w_qkv, w_dw),
            "vcol": vcol,
            "wpt": wpt,
            "id96": id96,
            "onesrow": np.ones((1, M96), np.float32),
        })

    res = run_bass_kernel_spmd(nc, in_maps, list(range(NCORES)), trace=_trace)

    y = np.empty((1, C, 256, 256), np.float32)
    for s in range(NCORES):
        blk = res.results[s]["y"].reshape(C, 4, 4, 8, GN)
        y[0, :, 32 * s:32 * (s + 1), :] = (
            blk.transpose(0, 3, 1, 4, 2).reshape(C, 32, 256))
    if _trace:
        return y, res
    return y


# revision 13
# speedup vs baseline: 1.8906x; 1.8906x over previous
"""Trainium2 Bass kernel for nn_AttentionV4 (patch attention, 8 heads on 8 cores).

Pipeline per core (= per head h):
  - The 1x1 qkv conv + depthwise 3x3 conv are fused into one dense 3x3 conv,
    expressed as a single matmul over a 6x6-windowed patch basis:
      Q/K/V[r, n] = sum_kappa W4[kappa, r] * Xp[kappa, n],
    kappa = (ph, pw, c) in [6,6,48] (1728, chunked 18 x 96), n = interior
    patch (64x64 grid = 4096; boundary patches of the stride-4 pad-4 unfold
    are exactly zero and are handled analytically).
  - unfold/fold with kernel=stride=4 are pure reshapes; Xp windows are read
    straight out of a host-prepared block-layout image xb[hm,wm,c,hq,wq].
  - l2-normalize Q (x temperature) and K per column, A = Qn^T Kn in [-1,1],
    so softmax needs no max subtraction: E = exp(A), Z = rowsum(E) + 260
    (260 = number of zero boundary K columns, each contributing exp(0)).
  - out = (V/Z) @ E accumulated over 32 row-tiles of 128.
  - fold + AllToAll routes output stripe s to core s; each core applies the
    final 48x48 projection to its own 32-row stripe.
"""
import sys
import types

sys.path.insert(0, "/opt/trn_rl_repo")

import numpy as np

# ---------------------------------------------------------------- constants
C = 48          # image channels
CH = 6          # channels per head
NH = 8          # heads == cores
GN = 64         # interior patch grid
N = GN * GN     # 4096 interior patches
M96 = 96        # rows of a head matrix (6ch * 4 * 4)
NKAP = 1728     # 36 windows * 48 channels
NCHUNK = 18     # kappa chunks of 96
ZCORR = 260.0   # 4356 - 4096 zero K-columns, exp(0) each
NPIECE = 16     # front-end N pieces (4 patch rows, 256 patches each)
NCORES = 8

# chunk table: pairs of (hm, wm), (hm, wm+1) groups within one (dh, dw) class
def _chunk_table():
    info = []
    for dh, dw in [(0, 0), (0, 1), (1, 0), (1, 1)]:
        groups = [(hm, wm)
                  for hm in range(4 if dh == 0 else 2)
                  for wm in range(4 if dw == 0 else 2)]
        for i in range(0, len(groups), 2):
            (hm0, wm0), (hm1, wm1) = groups[i], groups[i + 1]
            assert hm0 == hm1 and wm1 == wm0 + 1
            info.append((dh, dw, hm0, wm0))
    assert len(info) == NCHUNK
    return info

CHUNKS = _chunk_table()

# kappa order implied by the chunk table (ph, pw, c), c fastest
def _kappa_phpw():
    phs, pws = [], []
    for dh, dw, hm, wm0 in CHUNKS:
        for wm in (wm0, wm0 + 1):
            ph, pw = dh * 4 + hm, dw * 4 + wm
            phs += [ph] * C
            pws += [pw] * C
    return np.array(phs), np.array(pws)

_PHS, _PWS = _kappa_phpw()
_CS = np.tile(np.arange(C), NKAP // C)

# ---------------------------------------------------------------- host prep

def _build_xb(x):
    """Block layout of the (+1,+3)-padded image: xb[hm, wm, c, hq, wq]."""
    xpad = np.zeros((C, 260, 260), np.float32)
    xpad[:, 1:257, 1:257] = x[0]
    return np.ascontiguousarray(
        xpad.reshape(C, 65, 4, 65, 4).transpose(2, 4, 0, 1, 3))


def _build_w4(h, w_qkv, w_dw):
    """Fused (1x1 conv + dw3x3) weights in the kappa basis: [1728, 288]."""
    kh = np.arange(4)
    dy = _PHS[:, None] - kh[None, :]            # [1728, 4]
    dx = _PWS[:, None] - kh[None, :]
    my = (dy >= 0) & (dy < 3)
    mx = (dx >= 0) & (dx < 3)
    dyc = np.clip(dy, 0, 2)
    dxc = np.clip(dx, 0, 2)
    w4 = np.zeros((NKAP, 3, CH, 4, 4), np.float32)
    for sel in range(3):
        for cl in range(CH):
            o = sel * C + CH * h + cl
            wd = w_dw[o, 0]
            taps = (wd[dyc[:, :, None], dxc[:, None, :]]
                    * my[:, :, None] * mx[:, None, :])
            w4[:, sel, cl] = w_qkv[o, _CS][:, None, None] * taps
    return w4.reshape(NKAP, 288)


# ---------------------------------------------------------------- program

_PROG = None

def _build_program():
    import antenv  # noqa: F401
    if "antenv.axon_hooks" not in sys.modules:
        holder = {}
        m = types.ModuleType("antenv.axon_hooks")
        m.set_axon_ntff_profile_hook = lambda hk: holder.__setitem__("h", hk)
        m.get_axon_ntff_profile_hook = lambda: holder.get("h")
        sys.modules["antenv.axon_hooks"] = m
        antenv.axon_hooks = m
        try:
            from trn_agent_boot.trn_boot import _ntff_profile_via_ctypes
            m.set_axon_ntff_profile_hook(
                _ntff_profile_via_ctypes("/opt/axon/libaxon_pjrt.so"))
        except Exception:
            pass

    import concourse.bass as bass
    import concourse.tile as tile
    import concourse.mybir as mybir
    from contextlib import ExitStack

    F32 = mybir.dt.float32
    F32R = mybir.dt.float32r
    AF = mybir.ActivationFunctionType

    nc = bass.Bass("TRN2", num_devices=NCORES)

    xb_h = nc.dram_tensor("xb", [4, 4, C, 65, 65], F32R, kind="ExternalInput")
    w4_h = nc.dram_tensor("w4", [NKAP, 288], F32R, kind="ExternalInput")
    vcol_h = nc.dram_tensor("vcol", [M96, 2], F32R, kind="ExternalInput")
    wpt_h = nc.dram_tensor("wpt", [C, C], F32, kind="ExternalInput")
    id96_h = nc.dram_tensor("id96", [M96, M96], F32, kind="ExternalInput")
    ones_h = nc.dram_tensor("onesrow", [1, M96], F32, kind="ExternalInput")
    y_h = nc.dram_tensor("y", [C, 8192], F32, kind="ExternalOutput")
    cc_in = nc.dram_tensor("cc_in", [C, 8192], F32)
    cc_out = nc.dram_tensor("cc_out", [C, 8192], F32)

    with tile.TileContext(nc) as tc, ExitStack() as ctx, \
            nc.allow_low_precision(reason="float32r compute, fp32 tail"):
        const = ctx.enter_context(tc.tile_pool(name="const", bufs=1))
        w4_sb = const.tile([M96, NCHUNK * 288], F32R)
        nc.sync.dma_start(
            w4_sb[:].rearrange("p (k o) -> p k o", k=NCHUNK),
            w4_h[:].rearrange("(k p) o -> p k o", p=M96))
        vcol_sb = const.tile([M96, 2], F32R)
        nc.sync.dma_start(vcol_sb[:], vcol_h[:])
        wpt_sb = const.tile([C, C], F32)
        nc.sync.dma_start(wpt_sb[:], wpt_h[:])
        id96_sb = const.tile([M96, M96], F32)
        nc.sync.dma_start(id96_sb[:], id96_h[:])
        ones_sb = const.tile([1, M96], F32)
        nc.sync.dma_start(ones_sb[:], ones_h[:])

        persist = ctx.enter_context(tc.tile_pool(name="persist", bufs=1))
        qn = persist.tile([M96, N], F32R)
        kn = persist.tile([M96, N], F32R)
        vt = persist.tile([128, 32 * M96], F32)
        out_acc = persist.tile([M96, N], F32)
        zacc = persist.tile([128, 128], F32)
        rqt = persist.tile([128, 32], F32)

        # ---------------- front end: Q/K/V + column sumsq ----------------
        ctx2 = tc.tile_pool(name="fe_persist", bufs=1)
        fep = ctx2.__enter__()
        vn = fep.tile([M96, N], F32)
        rq_row = fep.tile([1, N], F32)
        rk_row = fep.tile([1, N], F32)
        with tc.tile_pool(name="fe_xp", bufs=3) as xp_pool, \
             tc.tile_pool(name="fe_ps", bufs=4, space="PSUM") as fps, \
             tc.tile_pool(name="fe_ssps", bufs=2, space="PSUM") as ssps, \
             tc.tile_pool(name="fe_tmp", bufs=2) as fe_tmp:
            for pp in range(NPIECE // 2):
                xps = []
                for half in range(2):
                    p = 2 * pp + half
                    r0 = 4 * p
                    xp_t = xp_pool.tile([M96, NCHUNK, 5, 65], F32R,
                                        name=f"xp{half}", tag="xp")
                    for k, (dh, dw, hm, wm0) in enumerate(CHUNKS):
                        nc.sync.dma_start(
                            xp_t[:, k, :, :],
                            xb_h[hm, wm0:wm0 + 2, :, r0:r0 + 5, :]
                            .rearrange("a c r w -> (a c) r w"))
                    xps.append(xp_t)
                for sel, dst in ((0, qn), (1, kn), (2, vn)):
                    pss = [fps.tile([M96, 256], F32, name=f"ps{half}", tag="ps")
                           for half in range(2)]
                    for k, (dh, dw, hm, wm0) in enumerate(CHUNKS):
                        for half in range(2):
                            nc.tensor.matmul(
                                pss[half][:],
                                lhsT=w4_sb[:, 288 * k + M96 * sel:
                                           288 * k + M96 * (sel + 1)],
                                rhs=xps[half][:, k, dh:dh + 4, dw:dw + 64],
                                start=(k == 0), stop=(k == NCHUNK - 1))
                    for half in range(2):
                        p = 2 * pp + half
                        cols = slice(256 * p, 256 * (p + 1))
                        nc.vector.tensor_copy(dst[:, cols], pss[half][:])
                        if sel < 2:
                            sq = fe_tmp.tile([M96, 256], F32R, name=f"sq{half}",
                                             tag="sq")
                            nc.scalar.activation(sq[:], pss[half][:], AF.Square)
                            ssp = ssps.tile([1, 256], F32, name=f"ssp{half}",
                                            tag="ssp")
                            nc.tensor.matmul(
                                ssp[:], lhsT=vcol_sb[:, sel:sel + 1], rhs=sq[:],
                                start=True, stop=True)
                            row = rq_row if sel == 0 else rk_row
                            nc.vector.tensor_copy(row[0:1, cols], ssp[:])

        # ---------------- rqt = rsqrt(sumsq_q) in row-tile layout ----------------
        with tc.tile_pool(name="rqps", bufs=1, space="PSUM") as rqpool:
            rqps = rqpool.tile([128, 32], F32)
            for t in range(32):
                nc.tensor.transpose(
                    rqps[:, t:t + 1], rq_row[0:1, 128 * t:128 * (t + 1)],
                    ones_sb[0:1, 0:1])
            nc.vector.reciprocal(rqt[:], rqps[:])
            nc.scalar.activation(rqt[:], rqt[:], AF.Sqrt)

        # ---------------- normalize K ----------------
        with tc.tile_pool(name="nrm", bufs=2) as npool, \
             tc.tile_pool(name="nrm_ps", bufs=2, space="PSUM") as npsum:
            for mt in range(8):
                cols = slice(512 * mt, 512 * (mt + 1))
                bp = npsum.tile([M96, 512], F32)
                nc.tensor.matmul(bp[:], lhsT=ones_sb[:], rhs=rk_row[0:1, cols],
                                 start=True, stop=True)
                b = npool.tile([M96, 512], F32)
                nc.vector.reciprocal(b[:], bp[:])
                nc.scalar.activation(b[:], b[:], AF.Sqrt)
                nc.vector.tensor_mul(kn[:, cols], kn[:, cols], b[:])

        # ---------------- V^T via PE transpose ----------------
        with tc.tile_pool(name="tp", bufs=2, space="PSUM") as tpool:
            for t in range(32):
                tp = tpool.tile([128, M96], F32)
                nc.tensor.transpose(
                    tp[:], vn[:, 128 * t:128 * (t + 1)], id96_sb[:])
                nc.vector.tensor_copy(vt[:, M96 * t:M96 * (t + 1)], tp[:])
        ctx2.__exit__(None, None, None)

        # ---------------- attention ----------------
        with tc.tile_pool(name="a_ps", bufs=2, space="PSUM") as apsum, \
             tc.tile_pool(name="o_ps", bufs=4, space="PSUM") as opsum, \
             tc.tile_pool(name="e_sb", bufs=6) as epool, \
             tc.tile_pool(name="z_sb", bufs=2) as zpool, \
             tc.tile_pool(name="vts", bufs=8) as vtspool:
            for g in range(8):
                estrips = []
                for tl in range(4):
                    t = 4 * g + tl
                    es = epool.tile([128, N], F32R)
                    estrips.append(es)
                    for mp in range(4):
                        pa = apsum.tile([128, 1024], F32)
                        for half in range(2):
                            nc.tensor.matmul(
                                pa[:, 512 * half:512 * (half + 1)],
                                lhsT=qn[:, 128 * t:128 * (t + 1)],
                                rhs=kn[:, 1024 * mp + 512 * half:
                                       1024 * mp + 512 * (half + 1)],
                                start=True, stop=True)
                        col = 4 * t + mp
                        nc.scalar.activation(
                            es[:, 1024 * mp:1024 * (mp + 1)], pa[:], AF.Exp,
                            scale=rqt[:, t:t + 1],
                            accum_out=zacc[:, col:col + 1])
                # Z for the group's 4 row-tiles: sum 4 accum cols, +260, 1/x
                zinv = zpool.tile([128, 4], F32)
                nc.vector.tensor_reduce(
                    zinv[:],
                    zacc[:, 16 * g:16 * (g + 1)].rearrange(
                        "p (t m) -> p t m", t=4),
                    axis=mybir.AxisListType.X, op=mybir.AluOpType.add)
                nc.vector.tensor_scalar_add(zinv[:], zinv[:], ZCORR)
                nc.vector.reciprocal(zinv[:], zinv[:])
                vts_tiles = []
                for tl in range(4):
                    t = 4 * g + tl
                    vts = vtspool.tile([128, M96], F32R)
                    nc.vector.tensor_scalar_mul(
                        vts[:], vt[:, M96 * t:M96 * (t + 1)],
                        zinv[:, tl:tl + 1])
                    vts_tiles.append(vts)
                for half in range(2):
                    pos = [opsum.tile([M96, 512], F32, tag="pos",
                                      name=f"pos{jj}")
                           for jj in range(4)]
                    for tl in range(4):
                        for jj in range(4):
                            j = 4 * half + jj
                            nc.tensor.matmul(
                                pos[jj][:], lhsT=vts_tiles[tl],
                                rhs=estrips[tl][:, 512 * j:512 * (j + 1)],
                                start=(tl == 0), stop=(tl == 3))
                    for jj in range(4):
                        j = 4 * half + jj
                        cols = slice(512 * j, 512 * (j + 1))
                        if g == 0:
                            nc.vector.tensor_copy(out_acc[:, cols], pos[jj][:])
                        else:
                            nc.vector.tensor_add(
                                out_acc[:, cols], out_acc[:, cols], pos[jj][:])

        # ---------------- fold staging + AllToAll + projection ----------------
        nc.sync.dma_start(
            cc_in[:].rearrange("(s cl) (khw i j) -> (cl khw) s i j",
                               s=8, cl=CH, khw=16, i=8),
            out_acc[:].rearrange("p (s i j) -> p s i j", s=8, i=8))
        nc.gpsimd.collective_compute(
            "AllToAll", mybir.AluOpType.bypass,
            replica_groups=[list(range(NCORES))],
            ins=[cc_in[:]], outs=[cc_out[:]])
        with tc.tile_pool(name="prj", bufs=2) as prj, \
             tc.tile_pool(name="prj_ps", bufs=2, space="PSUM") as prjps, \
             tc.tile_pool(name="yt", bufs=2) as ypool:
            for q in range(16):
                cols = slice(512 * q, 512 * (q + 1))
                fold_t = prj.tile([C, 512], F32)
                nc.sync.dma_start(fold_t[:], cc_out[:, cols])
                pp = prjps.tile([C, 512], F32)
                nc.tensor.matmul(pp[:], lhsT=wpt_sb[:], rhs=fold_t[:],
                                 start=True, stop=True)
                yt = ypool.tile([C, 512], F32)
                nc.vector.tensor_copy(yt[:], pp[:])
                nc.sync.dma_start(y_h[:, cols], yt[:])

    _split_excess_waits(nc)
    return nc


_wsplit_ctr = [0]

def _split_excess_waits(nc, max_waits=1):
    """This walrus build encodes only one sync-wait per instruction; hoist
    extras onto same-engine nops inserted directly before the instruction."""
    import bass_rust
    import concourse.mybir as mybir
    for fn in nc.m.functions:
        for bb in fn.blocks:
            insts = bb.instructions
            out = []
            changed = False
            for inst in insts:
                si = inst.sync_info
                if si is not None and len(si.on_wait) > max_waits:
                    waits = list(si.on_wait)
                    for w in waits[:-max_waits]:
                        _wsplit_ctr[0] += 1
                        nop = bass_rust.InstNoOp(
                            name=f"I-wsplit-{_wsplit_ctr[0]}", ins=[], outs=[])
                        nop.engine = inst.engine
                        nop.sync_info = mybir.SyncInfo(
                            on_wait=[w], on_update=[])
                        out.append(nop)
                    inst.sync_info = mybir.SyncInfo(
                        on_wait=waits[-max_waits:],
                        on_update=list(si.on_update))
                    changed = True
                out.append(inst)
            if changed:
                bb.instructions = out


def _get_program():
    global _PROG
    if _PROG is None:
        _PROG = _build_program()
    return _PROG


# ---------------------------------------------------------------- entry

def kernel(x, w_qkv, w_dw, temperature, w_proj, _trace=False):
    x = np.asarray(x, np.float32)
    w_qkv = np.asarray(w_qkv, np.float32)
    w_dw = np.asarray(w_dw, np.float32)
    temperature = np.asarray(temperature, np.float32)
    w_proj = np.asarray(w_proj, np.float32)

    nc = _get_program()
    from concourse.bass_utils import run_bass_kernel_spmd

    xb = _build_xb(x)
    id96 = np.eye(M96, dtype=np.float32)
    wpt = np.ascontiguousarray(w_proj.T)
    in_maps = []
    for h in range(NH):
        t_h = float(temperature[h, 0, 0])
        vcol = np.empty((M96, 2), np.float32)
        vcol[:, 0] = 1.0 / (t_h * t_h)
        vcol[:, 1] = 1.0
        in_maps.append({
            "xb": xb,
            "w4": _build_w4(h, w_qkv, w_dw),
            "vcol": vcol,
            "wpt": wpt,
            "id96": id96,
            "onesrow": np.ones((1, M96), np.float32),
        })

    res = run_bass_kernel_spmd(nc, in_maps, list(range(NCORES)), trace=_trace)

    y = np.empty((1, C, 256, 256), np.float32)
    for s in range(NCORES):
        blk = res.results[s]["y"].reshape(C, 4, 4, 8, GN)
        y[0, :, 32 * s:32 * (s + 1), :] = (
            blk.transpose(0, 3, 1, 4, 2).reshape(C, 32, 256))
    if _trace:
        return y, res
    return y
